# revision 20
# baseline (speedup 1.0000x reference)
"""Trainium2 8-core kernel for nn_ACCSLP_59485297050024.

The reference is a multiplicative-update NMF-style solver on N=4096 nodes with
rank R=128 and N_ITERS=2, returning a scalar objective O.

Because U, H, W, V are initialized to all-ones (per the problem's input spec),
every multiplicative update keeps each factor CONSTANT along the rank axis, so
the whole computation collapses exactly to rank-1 vector recurrences:

    u1 = (rowsum(S) + b*rowsum(Z)) * 2/(3R)
    h1 = (S + a*X)^T (1/e1) / R,  e1 = u1 + a       v1 = Z^T (1/u1) / R
    w1 = X (1/h1) / R,   u2 = (S + b*Z)(1/d1) / R,  d1 = h1 + b*v1
    h2 = (S + a*X)^T (1/e2) / R,  e2 = u2 + a*w1    v2 = Z^T (1/u2) / R
    w2 = X (1/h2) / R
    O  = R[Su2 Sh2 + a Sw2 Sh2 + b Su2 Sv2]
         - (sum(S) + a sum(X) + b sum(Z)) log R
         - <log u2, rsS + b rsZ> - a <log w2, rsX>
         - <log h2, csS + a csX> - b <log v2, csZ>

(verified exact vs the reference, rel err ~2e-16 in float64).

Device strategy (8 NeuronCores): row-shard S/X/Z (512 rows/core) and keep BOTH
the row-major shard and its transpose resident in SBUF (bf16, 2x96KB/partition).
All contractions are TensorE matmuls with tiny stationary vectors:
  - "B" passes (contract over rows) use the row-major tiles; partial results
    are AllReduce-summed across cores (2 AllReduces total).
  - "A" passes (contract over cols) use the transposed tiles; results stay
    core-local (each core owns its 512 rows of u/w).
The final scalar assembly (logs + dot products on 4096-vectors) runs on host.
"""

import numpy as np
import ml_dtypes

N = 4096
R = 128
ALPHA = 0.5
BETA = 0.5
N_CORES = 8
RPC = N // N_CORES          # rows per core = 512
RG = RPC // 128             # row groups per core = 4
NC_CH = N // 128            # 128-column chunks = 32
NJ8 = N // 512              # 512-column chunks = 8

_CACHED = {}


def _build():
    import concourse.mybir as mybir
    import concourse.tile as tile
    from concourse import bacc
    from concourse.masks import make_identity

    bf16 = mybir.dt.bfloat16
    f32 = mybir.dt.float32

    nc = bacc.Bacc("TRN2", target_bir_lowering=False, debug=False,
                   num_devices=N_CORES, dynamic_dma_scratch_size=8192)

    # per-core external I/O
    sr_e = nc.declare_dram_parameter("sr", [128, RG, N], bf16, isOutput=False)
    xr_e = nc.declare_dram_parameter("xr", [128, RG, N], bf16, isOutput=False)
    zr_e = nc.declare_dram_parameter("zr", [128, RG, N], bf16, isOutput=False)
    sc_e = nc.declare_dram_parameter("sc", [128, NC_CH, RPC], bf16, isOutput=False)
    xc_e = nc.declare_dram_parameter("xc", [128, NC_CH, RPC], bf16, isOutput=False)
    zc_e = nc.declare_dram_parameter("zc", [128, NC_CH, RPC], bf16, isOutput=False)
    out_u1 = nc.declare_dram_parameter("u1", [1, RPC], f32, isOutput=True)
    out_u2 = nc.declare_dram_parameter("u2", [1, RPC], f32, isOutput=True)
    out_w2 = nc.declare_dram_parameter("w2", [1, RPC], f32, isOutput=True)
    out_rsx = nc.declare_dram_parameter("rsx", [1, RPC], f32, isOutput=True)
    out_h2 = nc.declare_dram_parameter("h2", [NC_CH, 128], f32, isOutput=True)
    out_v2 = nc.declare_dram_parameter("v2", [NC_CH, 128], f32, isOutput=True)
    out_cssx = nc.declare_dram_parameter("cssx", [NC_CH, 128], f32, isOutput=True)
    out_csz = nc.declare_dram_parameter("csz", [NC_CH, 128], f32, isOutput=True)

    ar1_out = nc.dram_tensor("ar1_out", [4, NC_CH, 128], f32, addr_space="Shared")
    ar2_out = nc.dram_tensor("ar2_out", [2, NC_CH, 128], f32, addr_space="Shared")
    groups = [list(range(N_CORES))]

    with tile.TileContext(nc) as tc:
        with (
            tc.tile_pool(name="big", bufs=1) as big,
            tc.tile_pool(name="small", bufs=1) as small,
            tc.tile_pool(name="stg", bufs=2) as stg,
            tc.tile_pool(name="pp", bufs=1, space="PSUM") as pp,
            tc.tile_pool(name="pstream", bufs=2, space="PSUM") as pstream,
            tc.tile_pool(name="ptrans", bufs=1, space="PSUM") as ptrans,
            tc.tile_pool(name="dram", bufs=1, space="DRAM") as dram,
        ):
            # ---------- resident loads (pieces, ordered for pipelining) ----------
            QC = 4   # col-tile pieces per matrix (8 chunks each)
            QR = 4   # row-tile pieces per matrix (1024 cols each)
            tCs4 = [big.tile([128, 8, RPC], bf16, name=f"tC_s{q}", tag=f"tC_s{q}") for q in range(QC)]
            tCz4 = [big.tile([128, 8, RPC], bf16, name=f"tC_z{q}", tag=f"tC_z{q}") for q in range(QC)]
            tCx4 = [big.tile([128, 8, RPC], bf16, name=f"tC_x{q}", tag=f"tC_x{q}") for q in range(QC)]
            tRs4 = [big.tile([128, RG, N // QR], bf16, name=f"tR_s{q}", tag=f"tR_s{q}") for q in range(QR)]
            tRx4 = [big.tile([128, RG, N // QR], bf16, name=f"tR_x{q}", tag=f"tR_x{q}") for q in range(QR)]
            tRz4 = [big.tile([128, RG, N // QR], bf16, name=f"tR_z{q}", tag=f"tR_z{q}") for q in range(QR)]

            def tC(pieces, c):
                return pieces[c // 8][:, c % 8, :]

            def tR(pieces, a, c8):
                w = (c8 % 2) * RPC
                return pieces[c8 // 2][:, a, w:w + RPC]

            # cols of S/Z first (P0), then rows interleaved (B1), then cols of X (A2)
            for q in range(QC):
                nc.sync.dma_start(tCs4[q][:], sc_e[:, q * 8:(q + 1) * 8, :])
                nc.sync.dma_start(tCz4[q][:], zc_e[:, q * 8:(q + 1) * 8, :])

            # dummy AllReduce: aligns the 8 cores while DMA loads run, so the
            # real AllReduces later don't absorb the start-time skew.
            sync_in = dram.tile([1, 128], f32, tag="sync_in")
            sync_out = nc.dram_tensor("sync_out", [1, 128], f32, addr_space="Shared")
            zzero = small.tile([1, 128], f32, tag="zzero")
            with tc.high_priority():
                nc.gpsimd.memset(zzero[:], 0.0)
                nc.gpsimd.dma_start(sync_in[:], zzero[:])
                nc.gpsimd.collective_compute(
                    "AllReduce", mybir.AluOpType.add, replica_groups=groups,
                    ins=[sync_in.opt()], outs=[sync_out[:].opt()])

            W4 = N // QR
            for q in range(QR):
                nc.sync.dma_start(tRs4[q][:], sr_e[:, :, q * W4:(q + 1) * W4])
                nc.sync.dma_start(tRx4[q][:], xr_e[:, :, q * W4:(q + 1) * W4])
                nc.sync.dma_start(tRz4[q][:], zr_e[:, :, q * W4:(q + 1) * W4])
            for q in range(QC):
                nc.sync.dma_start(tCx4[q][:], xc_e[:, q * 8:(q + 1) * 8, :])

            ident = small.tile([128, 128], f32, tag="ident")
            make_identity(nc, ident[:])
            onesb = small.tile([128, 2], bf16, tag="onesb")
            nc.gpsimd.memset(onesb[:, 0:1], 1.0)
            nc.gpsimd.memset(onesb[:, 1:2], BETA)

            # ---------- P0: u1 = (rsS + b rsZ)/192 ----------
            ps_p0 = pp.tile([1, RPC], f32, tag="accA")
            for c in range(NC_CH):
                nc.tensor.matmul(ps_p0[:], onesb[:, 0:1], tC(tCs4, c),
                                 start=(c == 0), stop=False)
            for c in range(NC_CH):
                nc.tensor.matmul(ps_p0[:], onesb[:, 1:2], tC(tCz4, c),
                                 start=False, stop=(c == NC_CH - 1))
            u1sb = small.tile([1, RPC], f32, tag="u1sb")
            nc.vector.tensor_scalar_mul(u1sb[:], ps_p0[:], 2.0 / (3.0 * R))
            nc.sync.dma_start(out_u1[:], u1sb[:])

            # ---------- B1 stationaries: yS1=1/(R(u1+a)), yX1=a*yS1, yZ1=1/(R u1)
            y1S = small.tile([1, RPC], f32, tag="yS")
            y1X = small.tile([1, RPC], f32, tag="yX")
            y1Z = small.tile([1, RPC], f32, tag="yZ")
            t1 = small.tile([1, RPC], f32, tag="t1")
            nc.vector.tensor_scalar(t1[:], u1sb[:], ALPHA, float(R),
                                    mybir.AluOpType.add, mybir.AluOpType.mult)
            nc.vector.reciprocal(y1S[:], t1[:])
            nc.vector.tensor_scalar_mul(y1X[:], y1S[:], ALPHA)
            nc.vector.tensor_scalar_mul(t1[:], u1sb[:], float(R))
            nc.vector.reciprocal(y1Z[:], t1[:])

            def transpose_rows_to_cols(yvecs):
                """list of [1, 512] f32 -> psum [128, nvec*RG] grouped per vector."""
                ps_t = ptrans.tile([128, 96], f32, tag="pt")
                for v, yv in enumerate(yvecs):
                    for a in range(RG):
                        nc.tensor.transpose(
                            ps_t[:, v * RG + a: v * RG + a + 1],
                            yv[0:1, a * 128:(a + 1) * 128],
                            ident[0:1, 0:1],
                        )
                return ps_t

            ps_t1 = transpose_rows_to_cols([y1S, y1X, y1Z])
            statS1 = small.tile([128, 2, RG], bf16, tag="statS1")
            statX1 = small.tile([128, 2, RG], bf16, tag="statX1")
            statZ1 = small.tile([128, 2, RG], bf16, tag="statZ1")
            nc.gpsimd.memset(statS1[:, 1, :], 1.0)
            nc.gpsimd.memset(statX1[:, 1, :], ALPHA)
            nc.gpsimd.memset(statZ1[:, 1, :], 1.0)
            nc.vector.tensor_copy(statS1[:, 0, :], ps_t1[:, 0:RG])
            nc.vector.tensor_copy(statX1[:, 0, :], ps_t1[:, RG:2 * RG])
            nc.vector.tensor_copy(statZ1[:, 0, :], ps_t1[:, 2 * RG:3 * RG])

            # ---------- B1 (contract rows): h1pre,csSX | v1pre,csZ -> AR1 ----------
            ar1_in = dram.tile([4, NC_CH, 128], f32, tag="ar1_in")

            def b_pass(statS, statX, statZ, nv, ar_in_t):
                for c8 in range(NJ8):
                    ps_sx = pstream.tile([2, RPC], f32, tag="sx")
                    ps_z = pstream.tile([2, RPC], f32, tag="z")
                    for a in range(RG):
                        nc.tensor.matmul(ps_sx[0:nv, :], statS[:, :, a] if nv == 2 else statS[:, a:a + 1],
                                         tR(tRs4, a, c8), start=(a == 0), stop=False)
                    for a in range(RG):
                        nc.tensor.matmul(ps_sx[0:nv, :], statX[:, :, a] if nv == 2 else statX[:, a:a + 1],
                                         tR(tRx4, a, c8), start=False, stop=(a == RG - 1))
                    for a in range(RG):
                        nc.tensor.matmul(ps_z[0:nv, :], statZ[:, :, a] if nv == 2 else statZ[:, a:a + 1],
                                         tR(tRz4, a, c8), start=(a == 0), stop=(a == RG - 1))
                    st = stg.tile([34, RG, 128], f32, tag="st")
                    nc.scalar.copy(st[0:nv].rearrange("p a b -> p (a b)"), ps_sx[0:nv, :])
                    nc.vector.tensor_copy(st[32:32 + nv].rearrange("p a b -> p (a b)"), ps_z[0:nv, :])
                    nc.gpsimd.dma_start(ar_in_t[0:nv, c8 * RG:(c8 + 1) * RG, :], st[0:nv])
                    nc.gpsimd.dma_start(ar_in_t[nv:2 * nv, c8 * RG:(c8 + 1) * RG, :], st[32:32 + nv])

            b_pass(statS1, statX1, statZ1, 2, ar1_in)
            nc.gpsimd.collective_compute(
                "AllReduce", mybir.AluOpType.add, replica_groups=groups,
                ins=[ar1_in.opt()], outs=[ar1_out[:].opt()])

            # ---------- A2 stationaries from AR1 ----------
            h1 = small.tile([NC_CH, 128], f32, tag="h1")
            v1 = small.tile([NC_CH, 128], f32, tag="v1")
            nc.sync.dma_start(h1[:], ar1_out[0])
            nc.sync.dma_start(v1[:], ar1_out[2])
            vstk = small.tile([96, 128], f32, tag="vstk")
            tmp32 = small.tile([NC_CH, 128], f32, tag="tmp32")
            # invdS = 1/(R*(h1 + b*v1)); zhalf = b*invdS; invh1 = 1/(R*h1)
            nc.vector.tensor_scalar_mul(tmp32[:], v1[:], BETA)
            nc.vector.tensor_add(tmp32[:], tmp32[:], h1[:])
            nc.vector.tensor_scalar_mul(tmp32[:], tmp32[:], float(R))
            nc.vector.reciprocal(vstk[0:32, :], tmp32[:])
            nc.vector.tensor_scalar_mul(vstk[32:64, :], vstk[0:32, :], BETA)
            nc.vector.tensor_scalar_mul(tmp32[:], h1[:], float(R))
            nc.vector.reciprocal(vstk[64:96, :], tmp32[:])
            ps_t2 = ptrans.tile([128, 96], f32, tag="pt")
            nc.tensor.transpose(ps_t2[:], vstk[:], ident[0:96, 0:96])
            statA_sz = small.tile([128, 64], bf16, tag="statA_sz")
            statA_x = small.tile([128, 2, NC_CH], bf16, tag="statA_x")
            nc.vector.tensor_copy(statA_sz[:], ps_t2[:, 0:64])
            nc.gpsimd.memset(statA_x[:, 1, :], 1.0)
            nc.vector.tensor_copy(statA_x[:, 0, :], ps_t2[:, 64:96])

            # ---------- A2 (contract cols): u2 | w1, rsX ----------
            ps_a2u = pp.tile([1, RPC], f32, tag="accA")
            ps_a2wx = pp.tile([2, RPC], f32, tag="accB")
            for c in range(NC_CH):
                nc.tensor.matmul(ps_a2u[:], statA_sz[:, c:c + 1], tC(tCs4, c),
                                 start=(c == 0), stop=False)
            for c in range(NC_CH):
                nc.tensor.matmul(ps_a2u[:], statA_sz[:, 32 + c:33 + c], tC(tCz4, c),
                                 start=False, stop=(c == NC_CH - 1))
            for c in range(NC_CH):
                nc.tensor.matmul(ps_a2wx[:], statA_x[:, :, c], tC(tCx4, c),
                                 start=(c == 0), stop=(c == NC_CH - 1))
            a2u = small.tile([1, RPC], f32, tag="a2u")
            a2wx = small.tile([2, RPC], f32, tag="a2wx")
            nc.scalar.copy(a2u[:], ps_a2u[:])
            nc.scalar.copy(a2wx[:], ps_a2wx[:])
            nc.sync.dma_start(out_u2[:], a2u[:])
            nc.sync.dma_start(out_rsx[:], a2wx[1:2, :])

            # ---------- B2 stationaries: yS2=1/(R(u2+a w1)), yX2=a yS2, yZ2=1/(R u2)
            y2S = small.tile([1, RPC], f32, tag="yS")
            y2X = small.tile([1, RPC], f32, tag="yX")
            y2Z = small.tile([1, RPC], f32, tag="yZ")
            t2 = small.tile([1, RPC], f32, tag="t1")
            nc.vector.tensor_scalar_mul(t2[:], a2wx[0:1, :], ALPHA)
            nc.vector.tensor_add(t2[:], t2[:], a2u[:])
            nc.vector.tensor_scalar_mul(t2[:], t2[:], float(R))
            nc.vector.reciprocal(y2S[:], t2[:])
            nc.vector.tensor_scalar_mul(y2X[:], y2S[:], ALPHA)
            nc.vector.tensor_scalar_mul(t2[:], a2u[:], float(R))
            nc.vector.reciprocal(y2Z[:], t2[:])
            ps_t3 = transpose_rows_to_cols([y2S, y2X, y2Z])
            statS2 = small.tile([128, RG], bf16, tag="statS2")
            statX2 = small.tile([128, RG], bf16, tag="statX2")
            statZ2 = small.tile([128, RG], bf16, tag="statZ2")
            nc.vector.tensor_copy(statS2[:], ps_t3[:, 0:RG])
            nc.vector.tensor_copy(statX2[:], ps_t3[:, RG:2 * RG])
            nc.vector.tensor_copy(statZ2[:], ps_t3[:, 2 * RG:3 * RG])

            # ---------- B2 -> AR2 ----------
            ar2_in = dram.tile([2, NC_CH, 128], f32, tag="ar2_in")
            b_pass(statS2, statX2, statZ2, 1, ar2_in)
            nc.gpsimd.collective_compute(
                "AllReduce", mybir.AluOpType.add, replica_groups=groups,
                ins=[ar2_in.opt()], outs=[ar2_out[:].opt()])

            # ---------- A3: w2 = X (1/h2) / R ----------
            h2 = small.tile([NC_CH, 128], f32, tag="h1")
            nc.sync.dma_start(h2[:], ar2_out[0])
            nc.vector.tensor_scalar_mul(tmp32[:], h2[:], float(R))
            nc.vector.reciprocal(vstk[0:32, :], tmp32[:])
            ps_t4 = ptrans.tile([128, 96], f32, tag="pt")
            nc.tensor.transpose(ps_t4[:, 0:32], vstk[0:32, :], ident[0:32, 0:32])
            statA3 = small.tile([128, NC_CH], bf16, tag="statA3")
            nc.vector.tensor_copy(statA3[:], ps_t4[:, 0:32])
            ps_a3 = pp.tile([2, RPC], f32, tag="accB")
            for c in range(NC_CH):
                nc.tensor.matmul(ps_a3[0:1, :], statA3[:, c:c + 1], tC(tCx4, c),
                                 start=(c == 0), stop=(c == NC_CH - 1))
            w2sb = small.tile([1, RPC], f32, tag="u1sb")
            nc.scalar.copy(w2sb[:], ps_a3[0:1, :])
            nc.sync.dma_start(out_w2[:], w2sb[:])

            # ---------- replicated outputs (DRAM->DRAM) ----------
            nc.sync.dma_start(out_h2[:], ar2_out[0])
            nc.sync.dma_start(out_v2[:], ar2_out[1])
            nc.sync.dma_start(out_cssx[:], ar1_out[1])
            nc.sync.dma_start(out_csz[:], ar1_out[3])

    nc.compile()
    return nc


def _finale(res):
    """Assemble the scalar objective from per-core device outputs (float64)."""
    u1 = np.concatenate([np.asarray(res[i]["u1"], np.float64).ravel() for i in range(N_CORES)])
    u2 = np.concatenate([np.asarray(res[i]["u2"], np.float64).ravel() for i in range(N_CORES)])
    w2 = np.concatenate([np.asarray(res[i]["w2"], np.float64).ravel() for i in range(N_CORES)])
    rsx = np.concatenate([np.asarray(res[i]["rsx"], np.float64).ravel() for i in range(N_CORES)])
    h2 = np.asarray(res[0]["h2"], np.float64).ravel()
    v2 = np.asarray(res[0]["v2"], np.float64).ravel()
    cssx = np.asarray(res[0]["cssx"], np.float64).ravel()
    csz = np.asarray(res[0]["csz"], np.float64).ravel()

    rs_sz = u1 * (1.5 * R)          # = rsS + b*rsZ  (u1 = rs_sz*2/(3R))
    lR = np.log(R)
    term1 = R * (u2.sum() * h2.sum() + ALPHA * w2.sum() * h2.sum()
                 + BETA * u2.sum() * v2.sum())
    O = (term1
         - (rs_sz.sum() + ALPHA * rsx.sum()) * lR
         - (np.log(u2) * rs_sz).sum()
         - ALPHA * (np.log(w2) * rsx).sum()
         - (np.log(h2) * cssx).sum()
         - BETA * (np.log(v2) * csz).sum())
    return np.float32(O)


def _numpy_fallback(S, Z, X, U, H, W, V):
    """Faithful CPU implementation (only used if factors are not all-ones)."""
    S, Z, X, U, H, W, V = [np.asarray(a, np.float32) for a in (S, Z, X, U, H, W, V)]

    def obj(Sp, Xp, Zp):
        return ((Sp - S * np.log(Sp)).sum()
                + ALPHA * (Xp - X * np.log(Xp)).sum()
                + BETA * (Zp - Z * np.log(Zp)).sum())

    Sp = U @ H; Xp = W @ H; Zp = U @ V
    Sd = S / Sp; Xd = X / Xp; Zd = Z / Zp
    O = obj(Sp, Xp, Zp)
    for _ in range(2):
        dHV = H + BETA * V
        U = U * (Sd @ (H / dHV).T + Zd @ ((BETA * V) / dHV).T)
        Sp = U @ H; Zp = U @ V; Sd = S / Sp; Zd = Z / Zp
        dUW = U + ALPHA * W
        H = H * ((U / dUW).T @ Sd + ((ALPHA * W) / dUW).T @ Xd)
        Sp = U @ H; Xp = W @ H; Sd = S / Sp; Xd = X / Xp
        W = W * Xd.sum(axis=1, keepdims=True)
        Xp = W @ H; Xd = X / Xp
        V = V * Zd.sum(axis=0, keepdims=True)
        Zp = U @ V; Zd = Z / Zp
        O = obj(Sp, Xp, Zp)
    return np.float32(O)


def kernel(S, Z, X, U, H, W, V):
    if not (np.all(np.asarray(U) == 1) and np.all(np.asarray(H) == 1)
            and np.all(np.asarray(W) == 1) and np.all(np.asarray(V) == 1)):
        return _numpy_fallback(S, Z, X, U, H, W, V)

    from concourse.bass_utils import run_bass_kernel_spmd

    if "nc" not in _CACHED:
        _CACHED["nc"] = _build()
    nc = _CACHED["nc"]

    Sb = np.asarray(S, np.float32).astype(ml_dtypes.bfloat16)
    Xb = np.asarray(X, np.float32).astype(ml_dtypes.bfloat16)
    Zb = np.asarray(Z, np.float32).astype(ml_dtypes.bfloat16)

    def row_layout(shard):
        # [512, 4096] -> [128(p), RG(a), 4096(j)], per-partition contiguous
        return np.ascontiguousarray(shard.reshape(RG, 128, N).transpose(1, 0, 2))

    def col_layout(shard):
        # [512, 4096] -> [128(p), NC_CH(c), 512(l)] where (c,p) indexes column j
        return np.ascontiguousarray(
            shard.T.reshape(NC_CH, 128, RPC).transpose(1, 0, 2))

    in_maps = []
    for c in range(N_CORES):
        rows = slice(c * RPC, (c + 1) * RPC)
        sr_ = Sb[rows]; xr_ = Xb[rows]; zr_ = Zb[rows]
        in_maps.append({
            "sr": row_layout(sr_), "xr": row_layout(xr_), "zr": row_layout(zr_),
            "sc": col_layout(sr_), "xc": col_layout(xr_), "zc": col_layout(zr_),
        })

    res = run_bass_kernel_spmd(nc, in_maps, core_ids=list(range(N_CORES)))
    return _finale(res.results)


if __name__ == "__main__":
    import reference
    inputs = reference.setup_inputs()
    inputs = {k: np.asarray(v) for k, v in inputs.items()}
    print("kernel:", kernel(**inputs))


# revision 21
# speedup vs baseline: 1.0169x; 1.0169x over previous
"""Trainium2 8-core kernel for nn_ACCSLP_59485297050024.

The reference is a multiplicative-update NMF-style solver on N=4096 nodes with
rank R=128 and N_ITERS=2, returning a scalar objective O.

Because U, H, W, V are initialized to all-ones (per the problem's input spec),
every multiplicative update keeps each factor CONSTANT along the rank axis, so
the whole computation collapses exactly to rank-1 vector recurrences:

    u1 = (rowsum(S) + b*rowsum(Z)) * 2/(3R)
    h1 = (S + a*X)^T (1/e1) / R,  e1 = u1 + a       v1 = Z^T (1/u1) / R
    w1 = X (1/h1) / R,   u2 = (S + b*Z)(1/d1) / R,  d1 = h1 + b*v1
    h2 = (S + a*X)^T (1/e2) / R,  e2 = u2 + a*w1    v2 = Z^T (1/u2) / R
    w2 = X (1/h2) / R
    O  = R[Su2 Sh2 + a Sw2 Sh2 + b Su2 Sv2]
         - (sum(S) + a sum(X) + b sum(Z)) log R
         - <log u2, rsS + b rsZ> - a <log w2, rsX>
         - <log h2, csS + a csX> - b <log v2, csZ>

(verified exact vs the reference, rel err ~2e-16 in float64).

Device strategy (8 NeuronCores): row-shard S/X/Z (512 rows/core) and keep BOTH
the row-major shard and its transpose resident in SBUF (bf16, 2x96KB/partition).
All contractions are TensorE matmuls with tiny stationary vectors:
  - "B" passes (contract over rows) use the row-major tiles; partial results
    are AllReduce-summed across cores (2 AllReduces total).
  - "A" passes (contract over cols) use the transposed tiles; results stay
    core-local (each core owns its 512 rows of u/w).
The final scalar assembly (logs + dot products on 4096-vectors) runs on host.
"""

import numpy as np
import ml_dtypes

N = 4096
R = 128
ALPHA = 0.5
BETA = 0.5
N_CORES = 8
RPC = N // N_CORES          # rows per core = 512
RG = RPC // 128             # row groups per core = 4
NC_CH = N // 128            # 128-column chunks = 32
NJ8 = N // 512              # 512-column chunks = 8

_CACHED = {}


def _build():
    import concourse.mybir as mybir
    import concourse.tile as tile
    from concourse import bacc
    from concourse.masks import make_identity

    bf16 = mybir.dt.bfloat16
    f32 = mybir.dt.float32

    nc = bacc.Bacc("TRN2", target_bir_lowering=False, debug=False,
                   num_devices=N_CORES, dynamic_dma_scratch_size=8192)

    # per-core external I/O
    sr_e = nc.declare_dram_parameter("sr", [128, RG, N], bf16, isOutput=False)
    xr_e = nc.declare_dram_parameter("xr", [128, RG, N], bf16, isOutput=False)
    zr_e = nc.declare_dram_parameter("zr", [128, RG, N], bf16, isOutput=False)
    sc_e = nc.declare_dram_parameter("sc", [128, NC_CH, RPC], bf16, isOutput=False)
    xc_e = nc.declare_dram_parameter("xc", [128, NC_CH, RPC], bf16, isOutput=False)
    zc_e = nc.declare_dram_parameter("zc", [128, NC_CH, RPC], bf16, isOutput=False)
    out_u1 = nc.declare_dram_parameter("u1", [1, RPC], f32, isOutput=True)
    out_u2 = nc.declare_dram_parameter("u2", [1, RPC], f32, isOutput=True)
    out_w2 = nc.declare_dram_parameter("w2", [1, RPC], f32, isOutput=True)
    out_rsx = nc.declare_dram_parameter("rsx", [1, RPC], f32, isOutput=True)
    out_h2 = nc.declare_dram_parameter("h2", [NC_CH, 128], f32, isOutput=True)
    out_v2 = nc.declare_dram_parameter("v2", [NC_CH, 128], f32, isOutput=True)
    out_cssx = nc.declare_dram_parameter("cssx", [NC_CH, 128], f32, isOutput=True)
    out_csz = nc.declare_dram_parameter("csz", [NC_CH, 128], f32, isOutput=True)

    ar1_out = nc.dram_tensor("ar1_out", [4, NC_CH, 128], f32, addr_space="Shared")
    ar2_out = nc.dram_tensor("ar2_out", [2, NC_CH, 128], f32, addr_space="Shared")
    groups = [list(range(N_CORES))]

    with tile.TileContext(nc) as tc:
        with (
            tc.tile_pool(name="big", bufs=1) as big,
            tc.tile_pool(name="small", bufs=1) as small,
            tc.tile_pool(name="stg", bufs=2) as stg,
            tc.tile_pool(name="pp", bufs=1, space="PSUM") as pp,
            tc.tile_pool(name="pstream", bufs=2, space="PSUM") as pstream,
            tc.tile_pool(name="ptrans", bufs=1, space="PSUM") as ptrans,
            tc.tile_pool(name="dram", bufs=1, space="DRAM") as dram,
        ):
            # ---------- resident loads (pieces, ordered for pipelining) ----------
            QC = 4   # col-tile pieces per matrix (8 chunks each)
            QR = 4   # row-tile pieces per matrix (1024 cols each)
            tCs4 = [big.tile([128, 8, RPC], bf16, name=f"tC_s{q}", tag=f"tC_s{q}") for q in range(QC)]
            tCz4 = [big.tile([128, 8, RPC], bf16, name=f"tC_z{q}", tag=f"tC_z{q}") for q in range(QC)]
            tCx4 = [big.tile([128, 8, RPC], bf16, name=f"tC_x{q}", tag=f"tC_x{q}") for q in range(QC)]
            tRs4 = [big.tile([128, RG, N // QR], bf16, name=f"tR_s{q}", tag=f"tR_s{q}") for q in range(QR)]
            tRx4 = [big.tile([128, RG, N // QR], bf16, name=f"tR_x{q}", tag=f"tR_x{q}") for q in range(QR)]
            tRz4 = [big.tile([128, RG, N // QR], bf16, name=f"tR_z{q}", tag=f"tR_z{q}") for q in range(QR)]

            def tC(pieces, c):
                return pieces[c // 8][:, c % 8, :]

            def tR(pieces, a, c8):
                w = (c8 % 2) * RPC
                return pieces[c8 // 2][:, a, w:w + RPC]

            # cols of S/Z first (P0), then rows interleaved (B1), then cols of X (A2)
            for q in range(QC):
                nc.sync.dma_start(tCs4[q][:], sc_e[:, q * 8:(q + 1) * 8, :])
                nc.sync.dma_start(tCz4[q][:], zc_e[:, q * 8:(q + 1) * 8, :])

            # dummy AllReduce: aligns the 8 cores while DMA loads run, so the
            # real AllReduces later don't absorb the start-time skew.
            # input is never written (garbage) and output never read -- this is
            # purely a cross-core barrier, so it must not wait on any DMA.
            sync_in = nc.dram_tensor("sync_in", [1, 128], f32)
            sync_out = nc.dram_tensor("sync_out", [1, 128], f32, addr_space="Shared")
            with tc.high_priority():
                nc.gpsimd.collective_compute(
                    "AllReduce", mybir.AluOpType.add, replica_groups=groups,
                    ins=[sync_in[:].opt()], outs=[sync_out[:].opt()])

            W4 = N // QR
            for q in range(QR):
                nc.sync.dma_start(tRs4[q][:], sr_e[:, :, q * W4:(q + 1) * W4])
                nc.sync.dma_start(tRx4[q][:], xr_e[:, :, q * W4:(q + 1) * W4])
                nc.sync.dma_start(tRz4[q][:], zr_e[:, :, q * W4:(q + 1) * W4])
            for q in range(QC):
                nc.sync.dma_start(tCx4[q][:], xc_e[:, q * 8:(q + 1) * 8, :])

            ident = small.tile([128, 128], f32, tag="ident")
            make_identity(nc, ident[:])
            onesb = small.tile([128, 2], bf16, tag="onesb")
            nc.gpsimd.memset(onesb[:, 0:1], 1.0)
            nc.gpsimd.memset(onesb[:, 1:2], BETA)

            # ---------- P0: u1 = (rsS + b rsZ)/192 ----------
            ps_p0 = pp.tile([1, RPC], f32, tag="accA")
            for c in range(NC_CH):
                nc.tensor.matmul(ps_p0[:], onesb[:, 0:1], tC(tCs4, c),
                                 start=(c == 0), stop=False)
            for c in range(NC_CH):
                nc.tensor.matmul(ps_p0[:], onesb[:, 1:2], tC(tCz4, c),
                                 start=False, stop=(c == NC_CH - 1))
            u1sb = small.tile([1, RPC], f32, tag="u1sb")
            nc.vector.tensor_scalar_mul(u1sb[:], ps_p0[:], 2.0 / (3.0 * R))
            nc.sync.dma_start(out_u1[:], u1sb[:])

            # ---------- B1 stationaries: yS1=1/(R(u1+a)), yX1=a*yS1, yZ1=1/(R u1)
            y1S = small.tile([1, RPC], f32, tag="yS")
            y1X = small.tile([1, RPC], f32, tag="yX")
            y1Z = small.tile([1, RPC], f32, tag="yZ")
            t1 = small.tile([1, RPC], f32, tag="t1")
            nc.vector.tensor_scalar(t1[:], u1sb[:], ALPHA, float(R),
                                    mybir.AluOpType.add, mybir.AluOpType.mult)
            nc.vector.reciprocal(y1S[:], t1[:])
            nc.vector.tensor_scalar_mul(y1X[:], y1S[:], ALPHA)
            nc.vector.tensor_scalar_mul(t1[:], u1sb[:], float(R))
            nc.vector.reciprocal(y1Z[:], t1[:])

            def transpose_rows_to_cols(yvecs):
                """list of [1, 512] f32 -> psum [128, nvec*RG] grouped per vector."""
                ps_t = ptrans.tile([128, 96], f32, tag="pt")
                for v, yv in enumerate(yvecs):
                    for a in range(RG):
                        nc.tensor.transpose(
                            ps_t[:, v * RG + a: v * RG + a + 1],
                            yv[0:1, a * 128:(a + 1) * 128],
                            ident[0:1, 0:1],
                        )
                return ps_t

            ps_t1 = transpose_rows_to_cols([y1S, y1X, y1Z])
            statS1 = small.tile([128, 2, RG], bf16, tag="statS1")
            statX1 = small.tile([128, 2, RG], bf16, tag="statX1")
            statZ1 = small.tile([128, 2, RG], bf16, tag="statZ1")
            nc.gpsimd.memset(statS1[:, 1, :], 1.0)
            nc.gpsimd.memset(statX1[:, 1, :], ALPHA)
            nc.gpsimd.memset(statZ1[:, 1, :], 1.0)
            nc.vector.tensor_copy(statS1[:, 0, :], ps_t1[:, 0:RG])
            nc.vector.tensor_copy(statX1[:, 0, :], ps_t1[:, RG:2 * RG])
            nc.vector.tensor_copy(statZ1[:, 0, :], ps_t1[:, 2 * RG:3 * RG])

            # ---------- B1 (contract rows): h1pre,csSX | v1pre,csZ -> AR1 ----------
            ar1_in = dram.tile([4, NC_CH, 128], f32, tag="ar1_in")

            def b_pass(statS, statX, statZ, nv, ar_in_t):
                for c8 in range(NJ8):
                    ps_sx = pstream.tile([2, RPC], f32, tag="sx")
                    ps_z = pstream.tile([2, RPC], f32, tag="z")
                    for a in range(RG):
                        nc.tensor.matmul(ps_sx[0:nv, :], statS[:, :, a] if nv == 2 else statS[:, a:a + 1],
                                         tR(tRs4, a, c8), start=(a == 0), stop=False)
                    for a in range(RG):
                        nc.tensor.matmul(ps_sx[0:nv, :], statX[:, :, a] if nv == 2 else statX[:, a:a + 1],
                                         tR(tRx4, a, c8), start=False, stop=(a == RG - 1))
                    for a in range(RG):
                        nc.tensor.matmul(ps_z[0:nv, :], statZ[:, :, a] if nv == 2 else statZ[:, a:a + 1],
                                         tR(tRz4, a, c8), start=(a == 0), stop=(a == RG - 1))
                    st = stg.tile([34, RG, 128], f32, tag="st")
                    nc.scalar.copy(st[0:nv].rearrange("p a b -> p (a b)"), ps_sx[0:nv, :])
                    nc.vector.tensor_copy(st[32:32 + nv].rearrange("p a b -> p (a b)"), ps_z[0:nv, :])
                    nc.gpsimd.dma_start(ar_in_t[0:nv, c8 * RG:(c8 + 1) * RG, :], st[0:nv])
                    nc.gpsimd.dma_start(ar_in_t[nv:2 * nv, c8 * RG:(c8 + 1) * RG, :], st[32:32 + nv])

            b_pass(statS1, statX1, statZ1, 2, ar1_in)
            nc.gpsimd.collective_compute(
                "AllReduce", mybir.AluOpType.add, replica_groups=groups,
                ins=[ar1_in.opt()], outs=[ar1_out[:].opt()])

            # ---------- A2 stationaries from AR1 ----------
            h1 = small.tile([NC_CH, 128], f32, tag="h1")
            v1 = small.tile([NC_CH, 128], f32, tag="v1")
            nc.sync.dma_start(h1[:], ar1_out[0])
            nc.sync.dma_start(v1[:], ar1_out[2])
            vstk = small.tile([96, 128], f32, tag="vstk")
            tmp32 = small.tile([NC_CH, 128], f32, tag="tmp32")
            # invdS = 1/(R*(h1 + b*v1)); zhalf = b*invdS; invh1 = 1/(R*h1)
            nc.vector.tensor_scalar_mul(tmp32[:], v1[:], BETA)
            nc.vector.tensor_add(tmp32[:], tmp32[:], h1[:])
            nc.vector.tensor_scalar_mul(tmp32[:], tmp32[:], float(R))
            nc.vector.reciprocal(vstk[0:32, :], tmp32[:])
            nc.vector.tensor_scalar_mul(vstk[32:64, :], vstk[0:32, :], BETA)
            nc.vector.tensor_scalar_mul(tmp32[:], h1[:], float(R))
            nc.vector.reciprocal(vstk[64:96, :], tmp32[:])
            ps_t2 = ptrans.tile([128, 96], f32, tag="pt")
            nc.tensor.transpose(ps_t2[:], vstk[:], ident[0:96, 0:96])
            statA_sz = small.tile([128, 64], bf16, tag="statA_sz")
            statA_x = small.tile([128, 2, NC_CH], bf16, tag="statA_x")
            nc.vector.tensor_copy(statA_sz[:], ps_t2[:, 0:64])
            nc.gpsimd.memset(statA_x[:, 1, :], 1.0)
            nc.vector.tensor_copy(statA_x[:, 0, :], ps_t2[:, 64:96])

            # ---------- A2 (contract cols): u2 | w1, rsX ----------
            ps_a2u = pp.tile([1, RPC], f32, tag="accA")
            ps_a2wx = pp.tile([2, RPC], f32, tag="accB")
            for c in range(NC_CH):
                nc.tensor.matmul(ps_a2u[:], statA_sz[:, c:c + 1], tC(tCs4, c),
                                 start=(c == 0), stop=False)
            for c in range(NC_CH):
                nc.tensor.matmul(ps_a2u[:], statA_sz[:, 32 + c:33 + c], tC(tCz4, c),
                                 start=False, stop=(c == NC_CH - 1))
            for c in range(NC_CH):
                nc.tensor.matmul(ps_a2wx[:], statA_x[:, :, c], tC(tCx4, c),
                                 start=(c == 0), stop=(c == NC_CH - 1))
            a2u = small.tile([1, RPC], f32, tag="a2u")
            a2wx = small.tile([2, RPC], f32, tag="a2wx")
            nc.scalar.copy(a2u[:], ps_a2u[:])
            nc.scalar.copy(a2wx[:], ps_a2wx[:])
            nc.sync.dma_start(out_u2[:], a2u[:])
            nc.sync.dma_start(out_rsx[:], a2wx[1:2, :])

            # ---------- B2 stationaries: yS2=1/(R(u2+a w1)), yX2=a yS2, yZ2=1/(R u2)
            y2S = small.tile([1, RPC], f32, tag="yS")
            y2X = small.tile([1, RPC], f32, tag="yX")
            y2Z = small.tile([1, RPC], f32, tag="yZ")
            t2 = small.tile([1, RPC], f32, tag="t1")
            nc.vector.tensor_scalar_mul(t2[:], a2wx[0:1, :], ALPHA)
            nc.vector.tensor_add(t2[:], t2[:], a2u[:])
            nc.vector.tensor_scalar_mul(t2[:], t2[:], float(R))
            nc.vector.reciprocal(y2S[:], t2[:])
            nc.vector.tensor_scalar_mul(y2X[:], y2S[:], ALPHA)
            nc.vector.tensor_scalar_mul(t2[:], a2u[:], float(R))
            nc.vector.reciprocal(y2Z[:], t2[:])
            ps_t3 = transpose_rows_to_cols([y2S, y2X, y2Z])
            statS2 = small.tile([128, RG], bf16, tag="statS2")
            statX2 = small.tile([128, RG], bf16, tag="statX2")
            statZ2 = small.tile([128, RG], bf16, tag="statZ2")
            nc.vector.tensor_copy(statS2[:], ps_t3[:, 0:RG])
            nc.vector.tensor_copy(statX2[:], ps_t3[:, RG:2 * RG])
            nc.vector.tensor_copy(statZ2[:], ps_t3[:, 2 * RG:3 * RG])

            # ---------- B2 -> AR2 ----------
            ar2_in = dram.tile([2, NC_CH, 128], f32, tag="ar2_in")
            b_pass(statS2, statX2, statZ2, 1, ar2_in)
            nc.gpsimd.collective_compute(
                "AllReduce", mybir.AluOpType.add, replica_groups=groups,
                ins=[ar2_in.opt()], outs=[ar2_out[:].opt()])

            # ---------- A3: w2 = X (1/h2) / R ----------
            h2 = small.tile([NC_CH, 128], f32, tag="h1")
            nc.sync.dma_start(h2[:], ar2_out[0])
            nc.vector.tensor_scalar_mul(tmp32[:], h2[:], float(R))
            nc.vector.reciprocal(vstk[0:32, :], tmp32[:])
            ps_t4 = ptrans.tile([128, 96], f32, tag="pt")
            nc.tensor.transpose(ps_t4[:, 0:32], vstk[0:32, :], ident[0:32, 0:32])
            statA3 = small.tile([128, NC_CH], bf16, tag="statA3")
            nc.vector.tensor_copy(statA3[:], ps_t4[:, 0:32])
            ps_a3 = pp.tile([2, RPC], f32, tag="accB")
            for c in range(NC_CH):
                nc.tensor.matmul(ps_a3[0:1, :], statA3[:, c:c + 1], tC(tCx4, c),
                                 start=(c == 0), stop=(c == NC_CH - 1))
            w2sb = small.tile([1, RPC], f32, tag="u1sb")
            nc.scalar.copy(w2sb[:], ps_a3[0:1, :])
            nc.sync.dma_start(out_w2[:], w2sb[:])

            # ---------- replicated outputs (DRAM->DRAM) ----------
            nc.sync.dma_start(out_h2[:], ar2_out[0])
            nc.sync.dma_start(out_v2[:], ar2_out[1])
            nc.sync.dma_start(out_cssx[:], ar1_out[1])
            nc.sync.dma_start(out_csz[:], ar1_out[3])

    nc.compile()
    return nc


def _finale(res):
    """Assemble the scalar objective from per-core device outputs (float64)."""
    u1 = np.concatenate([np.asarray(res[i]["u1"], np.float64).ravel() for i in range(N_CORES)])
    u2 = np.concatenate([np.asarray(res[i]["u2"], np.float64).ravel() for i in range(N_CORES)])
    w2 = np.concatenate([np.asarray(res[i]["w2"], np.float64).ravel() for i in range(N_CORES)])
    rsx = np.concatenate([np.asarray(res[i]["rsx"], np.float64).ravel() for i in range(N_CORES)])
    h2 = np.asarray(res[0]["h2"], np.float64).ravel()
    v2 = np.asarray(res[0]["v2"], np.float64).ravel()
    cssx = np.asarray(res[0]["cssx"], np.float64).ravel()
    csz = np.asarray(res[0]["csz"], np.float64).ravel()

    rs_sz = u1 * (1.5 * R)          # = rsS + b*rsZ  (u1 = rs_sz*2/(3R))
    lR = np.log(R)
    term1 = R * (u2.sum() * h2.sum() + ALPHA * w2.sum() * h2.sum()
                 + BETA * u2.sum() * v2.sum())
    O = (term1
         - (rs_sz.sum() + ALPHA * rsx.sum()) * lR
         - (np.log(u2) * rs_sz).sum()
         - ALPHA * (np.log(w2) * rsx).sum()
         - (np.log(h2) * cssx).sum()
         - BETA * (np.log(v2) * csz).sum())
    return np.float32(O)


def _numpy_fallback(S, Z, X, U, H, W, V):
    """Faithful CPU implementation (only used if factors are not all-ones)."""
    S, Z, X, U, H, W, V = [np.asarray(a, np.float32) for a in (S, Z, X, U, H, W, V)]

    def obj(Sp, Xp, Zp):
        return ((Sp - S * np.log(Sp)).sum()
                + ALPHA * (Xp - X * np.log(Xp)).sum()
                + BETA * (Zp - Z * np.log(Zp)).sum())

    Sp = U @ H; Xp = W @ H; Zp = U @ V
    Sd = S / Sp; Xd = X / Xp; Zd = Z / Zp
    O = obj(Sp, Xp, Zp)
    for _ in range(2):
        dHV = H + BETA * V
        U = U * (Sd @ (H / dHV).T + Zd @ ((BETA * V) / dHV).T)
        Sp = U @ H; Zp = U @ V; Sd = S / Sp; Zd = Z / Zp
        dUW = U + ALPHA * W
        H = H * ((U / dUW).T @ Sd + ((ALPHA * W) / dUW).T @ Xd)
        Sp = U @ H; Xp = W @ H; Sd = S / Sp; Xd = X / Xp
        W = W * Xd.sum(axis=1, keepdims=True)
        Xp = W @ H; Xd = X / Xp
        V = V * Zd.sum(axis=0, keepdims=True)
        Zp = U @ V; Zd = Z / Zp
        O = obj(Sp, Xp, Zp)
    return np.float32(O)


def kernel(S, Z, X, U, H, W, V):
    if not (np.all(np.asarray(U) == 1) and np.all(np.asarray(H) == 1)
            and np.all(np.asarray(W) == 1) and np.all(np.asarray(V) == 1)):
        return _numpy_fallback(S, Z, X, U, H, W, V)

    from concourse.bass_utils import run_bass_kernel_spmd

    if "nc" not in _CACHED:
        _CACHED["nc"] = _build()
    nc = _CACHED["nc"]

    Sb = np.asarray(S, np.float32).astype(ml_dtypes.bfloat16)
    Xb = np.asarray(X, np.float32).astype(ml_dtypes.bfloat16)
    Zb = np.asarray(Z, np.float32).astype(ml_dtypes.bfloat16)

    def row_layout(shard):
        # [512, 4096] -> [128(p), RG(a), 4096(j)], per-partition contiguous
        return np.ascontiguousarray(shard.reshape(RG, 128, N).transpose(1, 0, 2))

    def col_layout(shard):
        # [512, 4096] -> [128(p), NC_CH(c), 512(l)] where (c,p) indexes column j
        return np.ascontiguousarray(
            shard.T.reshape(NC_CH, 128, RPC).transpose(1, 0, 2))

    in_maps = []
    for c in range(N_CORES):
        rows = slice(c * RPC, (c + 1) * RPC)
        sr_ = Sb[rows]; xr_ = Xb[rows]; zr_ = Zb[rows]
        in_maps.append({
            "sr": row_layout(sr_), "xr": row_layout(xr_), "zr": row_layout(zr_),
            "sc": col_layout(sr_), "xc": col_layout(xr_), "zc": col_layout(zr_),
        })

    res = run_bass_kernel_spmd(nc, in_maps, core_ids=list(range(N_CORES)))
    return _finale(res.results)


if __name__ == "__main__":
    import reference
    inputs = reference.setup_inputs()
    inputs = {k: np.asarray(v) for k, v in inputs.items()}
    print("kernel:", kernel(**inputs))
